# revision 24
# baseline (speedup 1.0000x reference)
"""Bass/Trainium2 kernel for GQA transformer block (nn_GQA_84353157694016).

Reference computation (B=2, S=2048, E=4096, H=32 q-heads, KVH=8 kv-heads, D=128):
    qkv = x @ wqkv.T                  -> split into q/k/v per GQA group
    q,k = rope_interleaved(q), rope_interleaved(k)
    out = softmax(causal(q k^T / sqrt(D))) @ v @ wo.T

Sharding (8 cores): pure tensor-parallel over kv groups — core t owns GQA
group t (4 q heads + its kv head, wqkv rows 768t:768(t+1)) and the matching
wo input rows 512t:512(t+1); each core runs BOTH batch elements
sequentially.  The partial outputs are summed on the host (the unshard
step of the reduce).

vs the TP4xDP2 predecessor: identical matmul column count, but both weight
matrices now fit resident in SBUF (wq 6.3MB + wo 4.2MB in fp16), removing
all mid-kernel weight streaming (~70MB less HBM traffic/core); fp16
replaces bf16 everywhere (same PE/DVE throughput, 8x lower error); the
attention-diagonal AV matmuls are column-trimmed; row-sum matmuls are
batched 8:1; startup loads are chunked across queues.

Layout strategy: everything on-chip is computed in "transposed" (feature x
sequence) orientation so the TensorE contraction dim always lands on
partitions with zero on-chip transposes (except v, which is PE-transposed).
Softmax is computed without max-subtraction, with exp biased by -2 so fp16
attn tiles stay in range; the row-sum is obtained by a ones-matmul over
8-tile DVE-accumulated groups in the same PSUM-accumulation pass as attn@v.
"""

import os
import sys

import numpy as np

for _p in ("/opt/trn_rl_repo",):
    if _p not in sys.path and os.path.isdir(_p):
        sys.path.append(_p)

import concourse.bass as bass
import concourse.tile as tile
from concourse import bacc, mybir
from concourse.bass_utils import run_bass_kernel_spmd
from concourse.masks import make_identity


def _install_ntff_hook():
    """bass_utils' trace path imports antenv.axon_hooks, which the agent image
    lacks; synthesize it (backed by trn_boot's ctypes NTFF driver) so
    trace=True / BASS_TRACE=1 works instead of crashing."""
    try:
        import antenv.axon_hooks  # noqa: F401
        return
    except ImportError:
        pass
    try:
        import types
        import antenv
        mod = types.ModuleType("antenv.axon_hooks")
        mod._hook = None
        mod.set_axon_ntff_profile_hook = lambda h: setattr(mod, "_hook", h)
        mod.get_axon_ntff_profile_hook = lambda: mod._hook
        sys.modules["antenv.axon_hooks"] = mod
        antenv.axon_hooks = mod
        from trn_agent_boot.trn_boot import _ntff_profile_via_ctypes
        so = "/opt/axon/libaxon_pjrt.so"
        if os.path.exists(so):
            mod._hook = _ntff_profile_via_ctypes(so)
    except Exception:
        pass


_install_ntff_hook()

# problem constants
B, S, E = 2, 2048, 4096
H, KVH, D = 32, 8, 128
QPK = H // KVH                    # 4 q heads per kv group
ROPE_BASE = 10000.0

NCORES = 8
TP = 8                            # tensor-parallel width (kv groups)

SC = 4                            # strips per batch
CW = S // SC                      # 512 strip width
NST = B * SC                      # 8 (batch, strip) pairs per core
NJT = (E + 2 * KVH * D) // TP // 128   # 6 qkv row-tiles (4 q + k + v)
NET = E // 128                    # 32 contraction tiles for qkv proj
HPC = H // TP                     # 4 q heads per core
FT = HPC * D // 128               # 4 local ctx feature tiles
ECN = E // CW                     # 8 output e-chunks
XCH = 8                           # x chunks per strip (NET/XCH et each)
ETC = NET // XCH                  # 8 et tiles per x chunk
EXP_BIAS = -2.0                   # keeps fp16 attn tiles < ~1.5k

f32 = mybir.dt.float32
fp16 = mybir.dt.float16
np_fp16 = np.float16

_built = {}


def _build_nc():
    nc = bacc.Bacc("TRN2", target_bir_lowering=False)

    xt_d = nc.dram_tensor("xt", [B, SC, 128, NET, CW], fp16, kind="ExternalInput")
    wq_d = nc.dram_tensor("wq", [NJT, 128, NET, 128], fp16, kind="ExternalInput")
    wo_d = nc.dram_tensor("wo", [ECN, 128, FT, CW], fp16, kind="ExternalInput")
    cq_d = nc.dram_tensor("cq", [128, S], fp16, kind="ExternalInput")
    sq_d = nc.dram_tensor("sq", [128, S], fp16, kind="ExternalInput")
    ck_d = nc.dram_tensor("ck", [128, S], fp16, kind="ExternalInput")
    sk_d = nc.dram_tensor("sk", [128, S], fp16, kind="ExternalInput")
    mk_d = nc.dram_tensor("mk", [128, 128], fp16, kind="ExternalInput")
    out_d = nc.dram_tensor("out", [B, SC, ECN, 128, SC, CW], fp16,
                           kind="ExternalOutput")

    with tile.TileContext(nc) as tc:
        with (
            tc.tile_pool(name="const", bufs=1) as constp,
            tc.tile_pool(name="wq", bufs=1) as wqp,
            tc.tile_pool(name="wo", bufs=1) as wop,
            tc.tile_pool(name="xt", bufs=10) as xtp,
            tc.tile_pool(name="st", bufs=8) as stp,
            tc.tile_pool(name="rt", bufs=2) as rtp,
            tc.tile_pool(name="q", bufs=2) as qp,
            tc.tile_pool(name="kv", bufs=1) as kvp,
            tc.tile_pool(name="at", bufs=8) as atp,
            tc.tile_pool(name="ata", bufs=3) as atap,
            tc.tile_pool(name="ctx", bufs=2) as ctxp,
            tc.tile_pool(name="ob", bufs=3) as obp,
            tc.tile_pool(name="rc", bufs=2) as rcp,
            # PSUM rings (8 banks total): proj/transpose 2, qk scores 2
            # (the AV->exp gating makes a 3rd qk buffer provably idle),
            # ctx+sums 2, wo-blocks 2.  Separate ctx/sums from the wo ring
            # so a new head's wo blocks never wait on the previous head's
            # softmax-normalization read of ctx_ps.
            tc.tile_pool(name="pmm", bufs=2, space="PSUM") as pmm,
            tc.tile_pool(name="pqk", bufs=3, space="PSUM") as pqk,
            tc.tile_pool(name="pacc", bufs=3, space="PSUM") as pacc,
        ):
            # ---- resident tensors, loaded once (chunked across queues) ----
            # The 16 DMA engines are shared across queues (~400GB/s
            # aggregate, ~130GB/s per busy hw queue; gpsimd's sw-DGE queue is
            # slower).  Interleave the startup-critical stream in first-use
            # order across all three queues so jt0's operands land first and
            # each later wq tile arrives just before the proj sweep needs it.
            wq_sb = [wqp.tile([128, NET, 128], fp16, tag=f"wq{j}",
                              name=f"wq{j}") for j in range(NJT)]
            wo_sb = [wop.tile([128, FT, CW], fp16, tag=f"wo{e}",
                              name=f"wo{e}") for e in range(ECN)]

            def load_wq(j, eng, hhalf):
                sl = slice(hhalf * (NET // 2), (hhalf + 1) * (NET // 2))
                eng.dma_start(out=wq_sb[j][:, sl, :], in_=wq_d[j, :, sl, :])

            x0h = [xtp.tile([128, ETC, CW], fp16, tag="xt", name="xc0")
                   for _ in range(XCH)]

            def load_x0(ch, eng):
                eng.dma_start(out=x0h[ch],
                              in_=xt_d[0, 0, :, ch * ETC:(ch + 1) * ETC, :])

            load_wq(0, nc.sync, 0)
            load_wq(0, nc.scalar, 1)
            load_x0(6, nc.gpsimd)
            load_x0(0, nc.sync)
            load_x0(1, nc.scalar)
            load_x0(7, nc.gpsimd)
            load_x0(2, nc.sync)
            load_x0(3, nc.scalar)
            load_wq(1, nc.gpsimd, 0)
            load_wq(1, nc.gpsimd, 1)
            load_x0(4, nc.sync)
            load_x0(5, nc.scalar)
            load_wq(2, nc.sync, 0)
            load_wq(2, nc.sync, 1)
            load_wq(3, nc.scalar, 0)
            load_wq(3, nc.scalar, 1)
            load_wq(4, nc.gpsimd, 0)
            load_wq(4, nc.gpsimd, 1)
            load_wq(5, nc.sync, 0)
            load_wq(5, nc.sync, 1)
            for e in range(ECN):
                nc.scalar.dma_start(out=wo_sb[e], in_=wo_d[e])
            cq_sb = constp.tile([128, S], fp16, tag="cq")
            sq_sb = constp.tile([128, S], fp16, tag="sq")
            ck_sb = constp.tile([128, S], fp16, tag="ck")
            sk_sb = constp.tile([128, S], fp16, tag="sk")
            mk_sb = constp.tile([128, 128], fp16, tag="mk")
            nc.gpsimd.dma_start(out=cq_sb, in_=cq_d[:, :])
            nc.gpsimd.dma_start(out=sq_sb, in_=sq_d[:, :])
            nc.gpsimd.dma_start(out=ck_sb, in_=ck_d[:, :])
            nc.gpsimd.dma_start(out=sk_sb, in_=sk_d[:, :])
            nc.gpsimd.dma_start(out=mk_sb, in_=mk_d[:, :])

            ident = constp.tile([128, 128], fp16, tag="ident")
            make_identity(nc, ident)
            ones_sb = constp.tile([128, 128], fp16, tag="ones")
            nc.vector.memset(ones_sb, 1.0)
            bias_sb = constp.tile([128, 1], f32, tag="bias")
            nc.vector.memset(bias_sb, EXP_BIAS)

            # persistent k (transposed) / v (natural), double-buffered by batch
            k_sb = [kvp.tile([128, S], fp16, tag=f"k{b}", name=f"k{b}")
                    for b in range(B)]
            v_sb = [kvp.tile([128, S // 128, 128], fp16, tag=f"v{b}",
                             name=f"v{b}") for b in range(B)]

            def emit_wo_block(bb, cs, ec, ctx_tiles):
                """Output-projection block: out[bb, strip cs, ec] = ctx @ woT."""
                ob = obp.tile([128, SC, CW], fp16, tag="ob", name="ob")
                for sti in range(SC):
                    ps = pacc.tile([128, CW], f32, tag="acc", name="wo_ps")
                    for ft in range(FT):
                        nc.tensor.matmul(
                            ps,
                            lhsT=ctx_tiles[:, ft, sti * 128:(sti + 1) * 128],
                            rhs=wo_sb[ec][:, ft, :],
                            start=(ft == 0),
                            stop=(ft == FT - 1),
                        )
                    # alternate the PSUM->SBUF cast between DVE and ACT
                    if sti % 2 == 0:
                        nc.vector.tensor_copy(ob[:, sti, :], ps)
                    else:
                        nc.scalar.copy(ob[:, sti, :], ps)
                    # half-granularity writeback on alternating queues keeps
                    # the final drain's output latency off the critical path
                    if sti == 1:
                        nc.sync.dma_start(out=out_d[bb, cs, ec, :, :2],
                                          in_=ob[:, :2, :])
                    elif sti == 3:
                        nc.scalar.dma_start(out=out_d[bb, cs, ec, :, 2:],
                                            in_=ob[:, 2:, :])

            for st in range(NST):
                b, c = divmod(st, SC)
                csl = slice(c * CW, (c + 1) * CW)

                # ---- x strip load, chunked (pipelines across strips) ----
                if st == 0:
                    xh = x0h
                else:
                    xh = []
                    for ch in range(XCH):
                        xc = xtp.tile([128, ETC, CW], fp16, tag="xt",
                                      name="xc")
                        eng = nc.sync if ch % 2 == 0 else nc.scalar
                        eng.dma_start(
                            out=xc,
                            in_=xt_d[b, c, :, ch * ETC:(ch + 1) * ETC, :])
                        xh.append(xc)

                # ---- fused QKV projection + RoPE + v transpose ----
                stage = []
                for jt in range(NJT):          # 4 q tiles, 1 k tile, 1 v tile
                    ps = pmm.tile([128, CW], f32, tag="mm", name="mm_ps")
                    for et in range(NET):
                        nc.tensor.matmul(
                            ps,
                            lhsT=wq_sb[jt][:, et, :],
                            rhs=xh[et // ETC][:, et % ETC, :],
                            start=(et == 0),
                            stop=(et == NET - 1),
                        )
                    stx = stp.tile([128, CW], fp16, tag="st", name="stx")
                    nc.scalar.copy(stx, ps)
                    stage.append(stx)

                q_sb = qp.tile([128, HPC, CW], fp16, tag="q")
                for sub in range(QPK + 1):     # RoPE on 4 q tiles + 1 k tile
                    stq = stage[sub]
                    is_q = sub < QPK
                    # interleaved pair-swap via partition-strided DMA
                    sw = rtp.tile([128, CW], fp16, tag="sw")
                    nc.sync.dma_start(out=sw[0::2, :], in_=stq[1::2, :])
                    nc.sync.dma_start(out=sw[1::2, :], in_=stq[0::2, :])
                    tmp = rtp.tile([128, CW], fp16, tag="rt")
                    nc.vector.tensor_mul(
                        tmp, sw, (sq_sb if is_q else sk_sb)[:, csl])
                    nc.vector.tensor_mul(
                        stq, stq, (cq_sb if is_q else ck_sb)[:, csl])
                    if is_q:
                        nc.vector.tensor_add(q_sb[:, sub, :], stq, tmp)
                    else:
                        nc.vector.tensor_add(k_sb[b][:, csl], stq, tmp)
                stv = stage[5]
                for u in range(CW // 128):
                    tp_ = pmm.tile([128, CW], fp16, tag="mm", name="tp_ps")
                    nc.tensor.transpose(
                        tp_[:, :128], stv[:, u * 128:(u + 1) * 128], ident
                    )
                    nc.scalar.copy(v_sb[b][:, (CW // 128) * c + u, :],
                                   tp_[:, :128])

                # ---- attention for strip (b, c), flash-style, no max ----
                njt2 = (CW // 128) * (c + 1)   # causal: k tiles 0..4c+3
                ctx_sb = ctxp.tile([128, HPC, CW], fp16, tag="ctx")
                for h in range(HPC):
                    if st > 0:
                        # software pipeline: prev strip's output-projection
                        # blocks (2 per head) fill PE while ACT/DVE run softmax
                        pb, pc = divmod(st - 1, SC)
                        emit_wo_block(pb, pc, 2 * h, prev_ctx)
                        emit_wo_block(pb, pc, 2 * h + 1, prev_ctx)
                    ctx_ps = pacc.tile([128, CW], f32, tag="acc")
                    sums_ps = pacc.tile([128, CW], f32, tag="acc")
                    at_acc = None
                    at_prev = None
                    for j2 in range(njt2):
                        # diagonal k-tiles: trim the fully-masked columns
                        # from the QK matmul, exp AND attn@v; zero-fill the
                        # attn tile so the row-sums stay full-width
                        diag = j2 >= njt2 - (CW // 128)
                        o = 128 * (j2 - (njt2 - (CW // 128))) if diag else 0
                        nw = CW - o
                        qk = pqk.tile([128, CW], f32, tag="qk")
                        nc.tensor.matmul(
                            qk[:, :nw],
                            lhsT=k_sb[b][:, j2 * 128:(j2 + 1) * 128],
                            rhs=q_sb[:, h, o:],
                            start=True, stop=True,
                        )
                        at = atp.tile([128, CW], fp16, tag="at")
                        if o:
                            nc.gpsimd.memset(at[:, :o], 0.0)
                        nc.scalar.activation(
                            at[:, o:], qk[:, :nw],
                            mybir.ActivationFunctionType.Exp,
                            bias=bias_sb,
                        )
                        if diag:
                            nc.vector.tensor_mul(
                                at[:, o:o + 128], at[:, o:o + 128], mk_sb
                            )
                        first, last = j2 == 0, j2 == njt2 - 1
                        nc.tensor.matmul(
                            ctx_ps[:, o:], lhsT=v_sb[b][:, j2, :],
                            rhs=at[:, o:],
                            start=first, stop=last,
                            skip_group_check=bool(o),
                        )
                        # batch the row-sum matmul over ALL attn tiles of
                        # the head: accumulate on DVE (fp16, peak value
                        # ~16*e^(9-2) stays in range), one ones-matmul total
                        if j2 == 0:
                            at_prev = at
                        elif j2 == 1:
                            at_acc = atap.tile([128, CW], fp16, tag="ata",
                                               name="at_acc")
                            nc.vector.tensor_add(at_acc, at_prev, at)
                        else:
                            nc.vector.tensor_add(at_acc, at_acc, at)
                        if last:
                            nc.tensor.matmul(
                                sums_ps,
                                lhsT=ones_sb,
                                rhs=at_acc if njt2 > 1 else at_prev,
                                start=True, stop=True,
                            )
                    rc = rcp.tile([128, CW], f32, tag="rc")
                    nc.vector.reciprocal_approx_fast(out=rc, in_=sums_ps)
                    nc.vector.tensor_mul(ctx_sb[:, h, :], ctx_ps, rc)

                prev_ctx = ctx_sb

            # drain: output projection for the final strip
            for ec in range(ECN):
                emit_wo_block(B - 1, SC - 1, ec, prev_ctx)
    nc.finalize()
    return nc


def _rope_tables(scale):
    inv = 1.0 / (ROPE_BASE ** (np.arange(0, D, 2, dtype=np.float64) / D))
    ang = np.arange(S, dtype=np.float64)[None, :] * inv[:, None]    # [D/2, S]
    C = np.empty((D, S), np.float32)
    Sx = np.empty((D, S), np.float32)
    C[0::2] = np.cos(ang)
    C[1::2] = np.cos(ang)
    Sx[0::2] = -np.sin(ang)
    Sx[1::2] = np.sin(ang)
    return (C * scale).astype(np_fp16), (Sx * scale).astype(np_fp16)


def _host_inputs(x, wqkv, wo):
    """Shard + retile inputs for the 8 cores. Core t = kv group t."""
    cq, sq = _rope_tables(D ** -0.5)
    ck, sk = _rope_tables(1.0)

    # causal boundary-block mask in scores^T layout: keep when jj <= ii
    jj = np.arange(128)[:, None]
    ii = np.arange(128)[None, :]
    mk = (jj <= ii).astype(np_fp16)

    xts = np.empty((B, SC, 128, NET, CW), np_fp16)
    for b in range(B):
        xT = np.ascontiguousarray(x[b].T)                 # [E, S]
        t = xT.reshape(NET, 128, SC, CW).transpose(2, 1, 0, 3)
        xts[b] = t.astype(np_fp16)

    rows = (E + 2 * KVH * D) // TP                        # 768
    in_maps = []
    for t in range(TP):
        wT = np.ascontiguousarray(wqkv[rows * t:rows * (t + 1)].T)  # [E, 768]
        wq_t = wT.reshape(NET, 128, NJT, 128).transpose(2, 1, 0, 3)
        woT = np.ascontiguousarray(wo[:, 512 * t:512 * (t + 1)].T)  # [512, E]
        wo_t = woT.reshape(FT, 128, ECN, CW).transpose(2, 1, 0, 3)
        in_maps.append({
            "xt": xts,
            "wq": np.ascontiguousarray(wq_t.astype(np_fp16)),
            "wo": np.ascontiguousarray(wo_t.astype(np_fp16)),
            "cq": cq, "sq": sq, "ck": ck, "sk": sk,
            "mk": mk,
        })
    return in_maps


def kernel(x, wqkv, wo):
    x = np.asarray(x, np.float32)
    wqkv = np.asarray(wqkv, np.float32)
    wo = np.asarray(wo, np.float32)

    if "nc" not in _built:
        _built["nc"] = _build_nc()
    nc = _built["nc"]

    in_maps = _host_inputs(x, wqkv, wo)
    res = run_bass_kernel_spmd(nc, in_maps, core_ids=list(range(NCORES)))
    globals()["_last_results"] = res

    acc = np.zeros((B, SC, ECN, 128, SC, CW), np.float32)
    for t in range(NCORES):
        acc += res.results[t]["out"].astype(np.float32)
    # [B, c, ec, p, sti, w] -> s = 512c + 128 sti + p, e = 512 ec + w
    out = acc.transpose(0, 1, 4, 3, 2, 5).reshape(B, S, E)
    return np.ascontiguousarray(out)


# revision 28
# speedup vs baseline: 1.0170x; 1.0170x over previous
"""Bass/Trainium2 kernel for GQA transformer block (nn_GQA_84353157694016).

Reference computation (B=2, S=2048, E=4096, H=32 q-heads, KVH=8 kv-heads, D=128):
    qkv = x @ wqkv.T                  -> split into q/k/v per GQA group
    q,k = rope_interleaved(q), rope_interleaved(k)
    out = softmax(causal(q k^T / sqrt(D))) @ v @ wo.T

Sharding (8 cores): pure tensor-parallel over kv groups — core t owns GQA
group t (4 q heads + its kv head, wqkv rows 768t:768(t+1)) and the matching
wo input rows 512t:512(t+1); each core runs BOTH batch elements
sequentially.  The partial outputs are summed on the host (the unshard
step of the reduce).

vs the TP4xDP2 predecessor: identical matmul column count, but both weight
matrices now fit resident in SBUF (wq 6.3MB + wo 4.2MB in fp16), removing
all mid-kernel weight streaming (~70MB less HBM traffic/core); fp16
replaces bf16 everywhere (same PE/DVE throughput, 8x lower error); the
attention-diagonal AV matmuls are column-trimmed; row-sum matmuls are
batched 8:1; startup loads are chunked across queues.

Layout strategy: everything on-chip is computed in "transposed" (feature x
sequence) orientation so the TensorE contraction dim always lands on
partitions with zero on-chip transposes (except v, which is PE-transposed).
Softmax is computed without max-subtraction, with exp biased by -2 so fp16
attn tiles stay in range; the row-sum is obtained by a ones-matmul over
8-tile DVE-accumulated groups in the same PSUM-accumulation pass as attn@v.
"""

import os
import sys

import numpy as np

for _p in ("/opt/trn_rl_repo",):
    if _p not in sys.path and os.path.isdir(_p):
        sys.path.append(_p)

import concourse.bass as bass
import concourse.tile as tile
from concourse import bacc, mybir
from concourse.bass_utils import run_bass_kernel_spmd
from concourse.masks import make_identity


def _install_ntff_hook():
    """bass_utils' trace path imports antenv.axon_hooks, which the agent image
    lacks; synthesize it (backed by trn_boot's ctypes NTFF driver) so
    trace=True / BASS_TRACE=1 works instead of crashing."""
    try:
        import antenv.axon_hooks  # noqa: F401
        return
    except ImportError:
        pass
    try:
        import types
        import antenv
        mod = types.ModuleType("antenv.axon_hooks")
        mod._hook = None
        mod.set_axon_ntff_profile_hook = lambda h: setattr(mod, "_hook", h)
        mod.get_axon_ntff_profile_hook = lambda: mod._hook
        sys.modules["antenv.axon_hooks"] = mod
        antenv.axon_hooks = mod
        from trn_agent_boot.trn_boot import _ntff_profile_via_ctypes
        so = "/opt/axon/libaxon_pjrt.so"
        if os.path.exists(so):
            mod._hook = _ntff_profile_via_ctypes(so)
    except Exception:
        pass


_install_ntff_hook()

# problem constants
B, S, E = 2, 2048, 4096
H, KVH, D = 32, 8, 128
QPK = H // KVH                    # 4 q heads per kv group
ROPE_BASE = 10000.0

NCORES = 8
TP = 8                            # tensor-parallel width (kv groups)

SC = 4                            # strips per batch
CW = S // SC                      # 512 strip width
NST = B * SC                      # 8 (batch, strip) pairs per core
NJT = (E + 2 * KVH * D) // TP // 128   # 6 qkv row-tiles (4 q + k + v)
NET = E // 128                    # 32 contraction tiles for qkv proj
HPC = H // TP                     # 4 q heads per core
FT = HPC * D // 128               # 4 local ctx feature tiles
ECN = E // CW                     # 8 output e-chunks
XCH = 8                           # x chunks per strip (NET/XCH et each)
ETC = NET // XCH                  # 8 et tiles per x chunk
EXP_BIAS = -2.0                   # keeps fp16 attn tiles < ~1.5k

f32 = mybir.dt.float32
fp16 = mybir.dt.float16
np_fp16 = np.float16

_built = {}


def _build_nc():
    nc = bacc.Bacc("TRN2", target_bir_lowering=False)

    xt_d = nc.dram_tensor("xt", [B, SC, 128, NET, CW], fp16, kind="ExternalInput")
    wq_d = nc.dram_tensor("wq", [NJT, 128, NET, 128], fp16, kind="ExternalInput")
    wo_d = nc.dram_tensor("wo", [ECN, 128, FT, CW], fp16, kind="ExternalInput")
    cq_d = nc.dram_tensor("cq", [128, S], fp16, kind="ExternalInput")
    sq_d = nc.dram_tensor("sq", [128, S], fp16, kind="ExternalInput")
    ck_d = nc.dram_tensor("ck", [128, S], fp16, kind="ExternalInput")
    sk_d = nc.dram_tensor("sk", [128, S], fp16, kind="ExternalInput")
    mk_d = nc.dram_tensor("mk", [128, 128], fp16, kind="ExternalInput")
    out_d = nc.dram_tensor("out", [B, SC, ECN, 128, SC, CW], fp16,
                           kind="ExternalOutput")

    with tile.TileContext(nc) as tc:
        with (
            tc.tile_pool(name="const", bufs=1) as constp,
            tc.tile_pool(name="wq", bufs=1) as wqp,
            tc.tile_pool(name="wo", bufs=1) as wop,
            tc.tile_pool(name="xt", bufs=10) as xtp,
            tc.tile_pool(name="st", bufs=8) as stp,
            tc.tile_pool(name="rt", bufs=2) as rtp,
            tc.tile_pool(name="q", bufs=2) as qp,
            tc.tile_pool(name="kv", bufs=1) as kvp,
            tc.tile_pool(name="at", bufs=8) as atp,
            tc.tile_pool(name="ata", bufs=3) as atap,
            tc.tile_pool(name="ctx", bufs=2) as ctxp,
            tc.tile_pool(name="ob", bufs=3) as obp,
            tc.tile_pool(name="rc", bufs=2) as rcp,
            # PSUM rings (8 banks total): proj/transpose 2, qk scores 2
            # (the AV->exp gating makes a 3rd qk buffer provably idle),
            # ctx+sums 2, wo-blocks 2.  Separate ctx/sums from the wo ring
            # so a new head's wo blocks never wait on the previous head's
            # softmax-normalization read of ctx_ps.
            tc.tile_pool(name="pmm", bufs=2, space="PSUM") as pmm,
            tc.tile_pool(name="pqk", bufs=3, space="PSUM") as pqk,
            tc.tile_pool(name="pacc", bufs=3, space="PSUM") as pacc,
        ):
            # ---- resident tensors, loaded once (chunked across queues) ----
            # The 16 DMA engines are shared across queues (~400GB/s
            # aggregate, ~130GB/s per busy hw queue; gpsimd's sw-DGE queue is
            # slower).  Interleave the startup-critical stream in first-use
            # order across all three queues so jt0's operands land first and
            # each later wq tile arrives just before the proj sweep needs it.
            wq_sb = [wqp.tile([128, NET, 128], fp16, tag=f"wq{j}",
                              name=f"wq{j}") for j in range(NJT)]
            wo_sb = [wop.tile([128, FT, CW], fp16, tag=f"wo{e}",
                              name=f"wo{e}") for e in range(ECN)]

            for j in range(NJT):
                for hhalf in range(2):
                    sl = slice(hhalf * (NET // 2), (hhalf + 1) * (NET // 2))
                    nc.gpsimd.dma_start(out=wq_sb[j][:, sl, :],
                                        in_=wq_d[j, :, sl, :])
            cq_sb = constp.tile([128, S], fp16, tag="cq")
            sq_sb = constp.tile([128, S], fp16, tag="sq")
            ck_sb = constp.tile([128, S], fp16, tag="ck")
            sk_sb = constp.tile([128, S], fp16, tag="sk")
            mk_sb = constp.tile([128, 128], fp16, tag="mk")
            nc.gpsimd.dma_start(out=cq_sb, in_=cq_d[:, :])
            nc.gpsimd.dma_start(out=sq_sb, in_=sq_d[:, :])
            nc.gpsimd.dma_start(out=ck_sb, in_=ck_d[:, :])
            nc.gpsimd.dma_start(out=sk_sb, in_=sk_d[:, :])
            nc.gpsimd.dma_start(out=mk_sb, in_=mk_d[:, :])
            for e in range(ECN):
                nc.gpsimd.dma_start(out=wo_sb[e], in_=wo_d[e])

            ident = constp.tile([128, 128], fp16, tag="ident")
            make_identity(nc, ident)
            ones_sb = constp.tile([128, 128], fp16, tag="ones")
            nc.vector.memset(ones_sb, 1.0)
            bias_sb = constp.tile([128, 1], f32, tag="bias")
            nc.vector.memset(bias_sb, EXP_BIAS)

            # persistent k (transposed) / v (natural), double-buffered by batch
            k_sb = [kvp.tile([128, S], fp16, tag=f"k{b}", name=f"k{b}")
                    for b in range(B)]
            v_sb = [kvp.tile([128, S // 128, 128], fp16, tag=f"v{b}",
                             name=f"v{b}") for b in range(B)]

            def emit_wo_block(bb, cs, ec, ctx_tiles):
                """Output-projection block: out[bb, strip cs, ec] = ctx @ woT."""
                ob = obp.tile([128, SC, CW], fp16, tag="ob", name="ob")
                for sti in range(SC):
                    # ride the qk ring (idle while wo blocks run) so ctx/sums
                    # keep pacc to themselves and head handoffs never stall
                    ps = pqk.tile([128, CW], f32, tag="qk", name="wo_ps")
                    for ft in range(FT):
                        nc.tensor.matmul(
                            ps,
                            lhsT=ctx_tiles[:, ft, sti * 128:(sti + 1) * 128],
                            rhs=wo_sb[ec][:, ft, :],
                            start=(ft == 0),
                            stop=(ft == FT - 1),
                        )
                    # alternate the PSUM->SBUF cast between DVE and ACT
                    if sti % 2 == 0:
                        nc.vector.tensor_copy(ob[:, sti, :], ps)
                    else:
                        nc.scalar.copy(ob[:, sti, :], ps)
                    # half-granularity writeback on alternating queues keeps
                    # the final drain's output latency off the critical path
                    if sti == 1:
                        nc.sync.dma_start(out=out_d[bb, cs, ec, :, :2],
                                          in_=ob[:, :2, :])
                    elif sti == 3:
                        nc.scalar.dma_start(out=out_d[bb, cs, ec, :, 2:],
                                            in_=ob[:, 2:, :])

            for st in range(NST):
                b, c = divmod(st, SC)
                csl = slice(c * CW, (c + 1) * CW)

                # ---- x strip load, chunked (pipelines across strips) ----
                xh = []
                for ch in range(XCH):
                    xc = xtp.tile([128, ETC, CW], fp16, tag="xt", name="xc")
                    eng = nc.sync if ch % 2 == 0 else nc.scalar
                    eng.dma_start(
                        out=xc,
                        in_=xt_d[b, c, :, ch * ETC:(ch + 1) * ETC, :])
                    xh.append(xc)

                # ---- fused QKV projection + RoPE + v transpose ----
                stage = []
                for jt in range(NJT):          # 4 q tiles, 1 k tile, 1 v tile
                    ps = pmm.tile([128, CW], f32, tag="mm", name="mm_ps")
                    for et in range(NET):
                        nc.tensor.matmul(
                            ps,
                            lhsT=wq_sb[jt][:, et, :],
                            rhs=xh[et // ETC][:, et % ETC, :],
                            start=(et == 0),
                            stop=(et == NET - 1),
                        )
                    stx = stp.tile([128, CW], fp16, tag="st", name="stx")
                    nc.scalar.copy(stx, ps)
                    stage.append(stx)

                q_sb = qp.tile([128, HPC, CW], fp16, tag="q")
                for sub in range(QPK + 1):     # RoPE on 4 q tiles + 1 k tile
                    stq = stage[sub]
                    is_q = sub < QPK
                    # interleaved pair-swap via partition-strided DMA
                    sw = rtp.tile([128, CW], fp16, tag="sw")
                    nc.sync.dma_start(out=sw[0::2, :], in_=stq[1::2, :])
                    nc.sync.dma_start(out=sw[1::2, :], in_=stq[0::2, :])
                    tmp = rtp.tile([128, CW], fp16, tag="rt")
                    nc.vector.tensor_mul(
                        tmp, sw, (sq_sb if is_q else sk_sb)[:, csl])
                    nc.vector.tensor_mul(
                        stq, stq, (cq_sb if is_q else ck_sb)[:, csl])
                    if is_q:
                        nc.vector.tensor_add(q_sb[:, sub, :], stq, tmp)
                    else:
                        nc.vector.tensor_add(k_sb[b][:, csl], stq, tmp)
                stv = stage[5]
                for u in range(CW // 128):
                    tp_ = pmm.tile([128, CW], fp16, tag="mm", name="tp_ps")
                    nc.tensor.transpose(
                        tp_[:, :128], stv[:, u * 128:(u + 1) * 128], ident
                    )
                    nc.scalar.copy(v_sb[b][:, (CW // 128) * c + u, :],
                                   tp_[:, :128])

                # ---- attention for strip (b, c), flash-style, no max ----
                njt2 = (CW // 128) * (c + 1)   # causal: k tiles 0..4c+3
                ctx_sb = ctxp.tile([128, HPC, CW], fp16, tag="ctx")
                for h in range(HPC):
                    if st > 0:
                        # software pipeline: prev strip's output-projection
                        # blocks (2 per head) fill PE while ACT/DVE run softmax
                        pb, pc = divmod(st - 1, SC)
                        emit_wo_block(pb, pc, 2 * h, prev_ctx)
                        emit_wo_block(pb, pc, 2 * h + 1, prev_ctx)
                    ctx_ps = pacc.tile([128, CW], f32, tag="acc")
                    sums_ps = pacc.tile([128, CW], f32, tag="acc")
                    at_acc = None
                    at_prev = None
                    for j2 in range(njt2):
                        # diagonal k-tiles: trim the fully-masked columns
                        # from the QK matmul, exp AND attn@v; zero-fill the
                        # attn tile so the row-sums stay full-width
                        diag = j2 >= njt2 - (CW // 128)
                        o = 128 * (j2 - (njt2 - (CW // 128))) if diag else 0
                        nw = CW - o
                        qk = pqk.tile([128, CW], f32, tag="qk")
                        nc.tensor.matmul(
                            qk[:, :nw],
                            lhsT=k_sb[b][:, j2 * 128:(j2 + 1) * 128],
                            rhs=q_sb[:, h, o:],
                            start=True, stop=True,
                        )
                        at = atp.tile([128, CW], fp16, tag="at")
                        if o:
                            nc.gpsimd.memset(at[:, :o], 0.0)
                        nc.scalar.activation(
                            at[:, o:], qk[:, :nw],
                            mybir.ActivationFunctionType.Exp,
                            bias=bias_sb,
                        )
                        if diag:
                            nc.vector.tensor_mul(
                                at[:, o:o + 128], at[:, o:o + 128], mk_sb
                            )
                        first, last = j2 == 0, j2 == njt2 - 1
                        nc.tensor.matmul(
                            ctx_ps[:, o:], lhsT=v_sb[b][:, j2, :],
                            rhs=at[:, o:],
                            start=first, stop=last,
                            skip_group_check=bool(o),
                        )
                        # batch the row-sum matmul over ALL attn tiles of
                        # the head: accumulate on DVE (fp16, peak value
                        # ~16*e^(9-2) stays in range), one ones-matmul total
                        if j2 == 0:
                            at_prev = at
                        elif j2 == 1:
                            at_acc = atap.tile([128, CW], fp16, tag="ata",
                                               name="at_acc")
                            nc.vector.tensor_add(at_acc, at_prev, at)
                        else:
                            nc.vector.tensor_add(at_acc, at_acc, at)
                        if last:
                            nc.tensor.matmul(
                                sums_ps,
                                lhsT=ones_sb,
                                rhs=at_acc if njt2 > 1 else at_prev,
                                start=True, stop=True,
                            )
                    rc = rcp.tile([128, CW], f32, tag="rc")
                    nc.vector.reciprocal_approx_fast(out=rc, in_=sums_ps)
                    nc.vector.tensor_mul(ctx_sb[:, h, :], ctx_ps, rc)

                prev_ctx = ctx_sb

            # drain: output projection for the final strip
            for ec in range(ECN):
                emit_wo_block(B - 1, SC - 1, ec, prev_ctx)
    nc.finalize()
    return nc


def _rope_tables(scale):
    inv = 1.0 / (ROPE_BASE ** (np.arange(0, D, 2, dtype=np.float64) / D))
    ang = np.arange(S, dtype=np.float64)[None, :] * inv[:, None]    # [D/2, S]
    C = np.empty((D, S), np.float32)
    Sx = np.empty((D, S), np.float32)
    C[0::2] = np.cos(ang)
    C[1::2] = np.cos(ang)
    Sx[0::2] = -np.sin(ang)
    Sx[1::2] = np.sin(ang)
    return (C * scale).astype(np_fp16), (Sx * scale).astype(np_fp16)


def _host_inputs(x, wqkv, wo):
    """Shard + retile inputs for the 8 cores. Core t = kv group t."""
    cq, sq = _rope_tables(D ** -0.5)
    ck, sk = _rope_tables(1.0)

    # causal boundary-block mask in scores^T layout: keep when jj <= ii
    jj = np.arange(128)[:, None]
    ii = np.arange(128)[None, :]
    mk = (jj <= ii).astype(np_fp16)

    xts = np.empty((B, SC, 128, NET, CW), np_fp16)
    for b in range(B):
        xT = np.ascontiguousarray(x[b].T)                 # [E, S]
        t = xT.reshape(NET, 128, SC, CW).transpose(2, 1, 0, 3)
        xts[b] = t.astype(np_fp16)

    rows = (E + 2 * KVH * D) // TP                        # 768
    in_maps = []
    for t in range(TP):
        wT = np.ascontiguousarray(wqkv[rows * t:rows * (t + 1)].T)  # [E, 768]
        wq_t = wT.reshape(NET, 128, NJT, 128).transpose(2, 1, 0, 3)
        woT = np.ascontiguousarray(wo[:, 512 * t:512 * (t + 1)].T)  # [512, E]
        wo_t = woT.reshape(FT, 128, ECN, CW).transpose(2, 1, 0, 3)
        in_maps.append({
            "xt": xts,
            "wq": np.ascontiguousarray(wq_t.astype(np_fp16)),
            "wo": np.ascontiguousarray(wo_t.astype(np_fp16)),
            "cq": cq, "sq": sq, "ck": ck, "sk": sk,
            "mk": mk,
        })
    return in_maps


def kernel(x, wqkv, wo):
    x = np.asarray(x, np.float32)
    wqkv = np.asarray(wqkv, np.float32)
    wo = np.asarray(wo, np.float32)

    if "nc" not in _built:
        _built["nc"] = _build_nc()
    nc = _built["nc"]

    in_maps = _host_inputs(x, wqkv, wo)
    res = run_bass_kernel_spmd(nc, in_maps, core_ids=list(range(NCORES)))
    globals()["_last_results"] = res

    acc = np.zeros((B, SC, ECN, 128, SC, CW), np.float32)
    for t in range(NCORES):
        acc += res.results[t]["out"].astype(np.float32)
    # [B, c, ec, p, sti, w] -> s = 512c + 128 sti + p, e = 512 ec + w
    out = acc.transpose(0, 1, 4, 3, 2, 5).reshape(B, S, E)
    return np.ascontiguousarray(out)


# revision 34
# speedup vs baseline: 1.0200x; 1.0030x over previous
"""Bass/Trainium2 kernel for GQA transformer block (nn_GQA_84353157694016).

Reference computation (B=2, S=2048, E=4096, H=32 q-heads, KVH=8 kv-heads, D=128):
    qkv = x @ wqkv.T                  -> split into q/k/v per GQA group
    q,k = rope_interleaved(q), rope_interleaved(k)
    out = softmax(causal(q k^T / sqrt(D))) @ v @ wo.T

Sharding (8 cores): pure tensor-parallel over kv groups — core t owns GQA
group t (4 q heads + its kv head, wqkv rows 768t:768(t+1)) and the matching
wo input rows 512t:512(t+1); each core runs BOTH batch elements
sequentially.  The partial outputs are summed on the host (the unshard
step of the reduce).

vs the TP4xDP2 predecessor: identical matmul column count, but both weight
matrices now fit resident in SBUF (wq 6.3MB + wo 4.2MB in fp16), removing
all mid-kernel weight streaming (~70MB less HBM traffic/core); fp16
replaces bf16 everywhere (same PE/DVE throughput, 8x lower error); the
attention-diagonal AV matmuls are column-trimmed; row-sum matmuls are
batched 8:1; startup loads are chunked across queues.

Layout strategy: everything on-chip is computed in "transposed" (feature x
sequence) orientation so the TensorE contraction dim always lands on
partitions with zero on-chip transposes (except v, which is PE-transposed).
Softmax is computed without max-subtraction, with exp biased by -2 so fp16
attn tiles stay in range; the row-sum is obtained by a ones-matmul over
8-tile DVE-accumulated groups in the same PSUM-accumulation pass as attn@v.
"""

import os
import sys

import numpy as np

for _p in ("/opt/trn_rl_repo",):
    if _p not in sys.path and os.path.isdir(_p):
        sys.path.append(_p)

import concourse.bass as bass
import concourse.tile as tile
from concourse import bacc, mybir
from concourse.bass_utils import run_bass_kernel_spmd
from concourse.masks import make_identity


def _install_ntff_hook():
    """bass_utils' trace path imports antenv.axon_hooks, which the agent image
    lacks; synthesize it (backed by trn_boot's ctypes NTFF driver) so
    trace=True / BASS_TRACE=1 works instead of crashing."""
    try:
        import antenv.axon_hooks  # noqa: F401
        return
    except ImportError:
        pass
    try:
        import types
        import antenv
        mod = types.ModuleType("antenv.axon_hooks")
        mod._hook = None
        mod.set_axon_ntff_profile_hook = lambda h: setattr(mod, "_hook", h)
        mod.get_axon_ntff_profile_hook = lambda: mod._hook
        sys.modules["antenv.axon_hooks"] = mod
        antenv.axon_hooks = mod
        from trn_agent_boot.trn_boot import _ntff_profile_via_ctypes
        so = "/opt/axon/libaxon_pjrt.so"
        if os.path.exists(so):
            mod._hook = _ntff_profile_via_ctypes(so)
    except Exception:
        pass


_install_ntff_hook()

# problem constants
B, S, E = 2, 2048, 4096
H, KVH, D = 32, 8, 128
QPK = H // KVH                    # 4 q heads per kv group
ROPE_BASE = 10000.0

NCORES = 8
TP = 8                            # tensor-parallel width (kv groups)

SC = 4                            # strips per batch
CW = S // SC                      # 512 strip width
NST = B * SC                      # 8 (batch, strip) pairs per core
NJT = (E + 2 * KVH * D) // TP // 128   # 6 qkv row-tiles (4 q + k + v)
NET = E // 128                    # 32 contraction tiles for qkv proj
HPC = H // TP                     # 4 q heads per core
FT = HPC * D // 128               # 4 local ctx feature tiles
ECN = E // CW                     # 8 output e-chunks
XCH = 8                           # x chunks per strip (NET/XCH et each)
ETC = NET // XCH                  # 8 et tiles per x chunk
EXP_BIAS = -2.0                   # keeps fp16 attn tiles < ~1.5k

f32 = mybir.dt.float32
fp16 = mybir.dt.float16
np_fp16 = np.float16

_built = {}


def _build_nc():
    nc = bacc.Bacc("TRN2", target_bir_lowering=False)

    xt_d = nc.dram_tensor("xt", [B, SC, 128, NET, CW], fp16, kind="ExternalInput")
    wq_d = nc.dram_tensor("wq", [NJT, 128, NET, 128], fp16, kind="ExternalInput")
    wo_d = nc.dram_tensor("wo", [ECN, 128, FT, CW], fp16, kind="ExternalInput")
    cq_d = nc.dram_tensor("cq", [128, S], fp16, kind="ExternalInput")
    sq_d = nc.dram_tensor("sq", [128, S], fp16, kind="ExternalInput")
    ck_d = nc.dram_tensor("ck", [128, S], fp16, kind="ExternalInput")
    sk_d = nc.dram_tensor("sk", [128, S], fp16, kind="ExternalInput")
    mk_d = nc.dram_tensor("mk", [128, 128], fp16, kind="ExternalInput")
    out_d = nc.dram_tensor("out", [B, SC, ECN, 128, SC, CW], fp16,
                           kind="ExternalOutput")

    with tile.TileContext(nc) as tc:
        with (
            tc.tile_pool(name="const", bufs=1) as constp,
            tc.tile_pool(name="wq", bufs=1) as wqp,
            tc.tile_pool(name="wo", bufs=1) as wop,
            tc.tile_pool(name="xt", bufs=10) as xtp,
            tc.tile_pool(name="st", bufs=8) as stp,
            tc.tile_pool(name="rt", bufs=2) as rtp,
            tc.tile_pool(name="q", bufs=2) as qp,
            tc.tile_pool(name="kv", bufs=1) as kvp,
            tc.tile_pool(name="at", bufs=8) as atp,
            tc.tile_pool(name="ata", bufs=3) as atap,
            tc.tile_pool(name="ctx", bufs=2) as ctxp,
            tc.tile_pool(name="ob", bufs=3) as obp,
            tc.tile_pool(name="rc", bufs=2) as rcp,
            # PSUM rings (8 banks total): proj/transpose 2, qk scores 2
            # (the AV->exp gating makes a 3rd qk buffer provably idle),
            # ctx+sums 2, wo-blocks 2.  Separate ctx/sums from the wo ring
            # so a new head's wo blocks never wait on the previous head's
            # softmax-normalization read of ctx_ps.
            tc.tile_pool(name="pmm", bufs=2, space="PSUM") as pmm,
            tc.tile_pool(name="pqk", bufs=3, space="PSUM") as pqk,
            tc.tile_pool(name="pacc", bufs=3, space="PSUM") as pacc,
        ):
            # ---- resident tensors, loaded once (chunked across queues) ----
            # The 16 DMA engines are shared across queues (~400GB/s
            # aggregate, ~130GB/s per busy hw queue; gpsimd's sw-DGE queue is
            # slower).  Interleave the startup-critical stream in first-use
            # order across all three queues so jt0's operands land first and
            # each later wq tile arrives just before the proj sweep needs it.
            wq_sb = [wqp.tile([128, NET, 128], fp16, tag=f"wq{j}",
                              name=f"wq{j}") for j in range(NJT)]
            wo_sb = [wop.tile([128, FT, CW], fp16, tag=f"wo{e}",
                              name=f"wo{e}") for e in range(ECN)]

            for j in range(NJT):
                for hhalf in range(2):
                    sl = slice(hhalf * (NET // 2), (hhalf + 1) * (NET // 2))
                    nc.gpsimd.dma_start(out=wq_sb[j][:, sl, :],
                                        in_=wq_d[j, :, sl, :])
            cq_sb = constp.tile([128, S], fp16, tag="cq")
            sq_sb = constp.tile([128, S], fp16, tag="sq")
            ck_sb = constp.tile([128, S], fp16, tag="ck")
            sk_sb = constp.tile([128, S], fp16, tag="sk")
            mk_sb = constp.tile([128, 128], fp16, tag="mk")
            nc.gpsimd.dma_start(out=mk_sb, in_=mk_d[:, :])
            for e in range(ECN):
                nc.gpsimd.dma_start(out=wo_sb[e], in_=wo_d[e])

            ident = constp.tile([128, 128], fp16, tag="ident")
            make_identity(nc, ident)
            ones_sb = constp.tile([128, 128], fp16, tag="ones")
            nc.vector.memset(ones_sb, 1.0)
            bias_sb = constp.tile([128, 1], f32, tag="bias")
            nc.vector.memset(bias_sb, EXP_BIAS)

            # persistent k (transposed) / v (natural), double-buffered by batch
            k_sb = [kvp.tile([128, S], fp16, tag=f"k{b}", name=f"k{b}")
                    for b in range(B)]
            v_sb = [kvp.tile([128, S // 128, 128], fp16, tag=f"v{b}",
                             name=f"v{b}") for b in range(B)]

            def emit_wo_block(bb, cs, ec, ctx_tiles):
                """Output-projection block: out[bb, strip cs, ec] = ctx @ woT."""
                ob = obp.tile([128, SC, CW], fp16, tag="ob", name="ob")
                for sti in range(SC):
                    # ride the qk ring (idle while wo blocks run) so ctx/sums
                    # keep pacc to themselves and head handoffs never stall
                    ps = pqk.tile([128, CW], f32, tag="qk", name="wo_ps")
                    for ft in range(FT):
                        nc.tensor.matmul(
                            ps,
                            lhsT=ctx_tiles[:, ft, sti * 128:(sti + 1) * 128],
                            rhs=wo_sb[ec][:, ft, :],
                            start=(ft == 0),
                            stop=(ft == FT - 1),
                        )
                    # alternate the PSUM->SBUF cast between DVE and ACT
                    if sti % 2 == 0:
                        nc.vector.tensor_copy(ob[:, sti, :], ps)
                    else:
                        nc.scalar.copy(ob[:, sti, :], ps)
                    # half-granularity writeback on alternating queues keeps
                    # the final drain's output latency off the critical path
                    if sti == 1:
                        nc.sync.dma_start(out=out_d[bb, cs, ec, :, :2],
                                          in_=ob[:, :2, :])
                    elif sti == 3:
                        nc.scalar.dma_start(out=out_d[bb, cs, ec, :, 2:],
                                            in_=ob[:, 2:, :])

            for st in range(NST):
                b, c = divmod(st, SC)
                csl = slice(c * CW, (c + 1) * CW)

                # ---- x strip load, chunked (pipelines across strips) ----
                if st == 1:
                    xh = x1h
                else:
                    xh = []
                    for ch in range(XCH):
                        xc = xtp.tile([128, ETC, CW], fp16, tag="xt",
                                      name="xc")
                        eng = nc.sync if ch % 2 == 0 else nc.scalar
                        eng.dma_start(
                            out=xc,
                            in_=xt_d[b, c, :, ch * ETC:(ch + 1) * ETC, :])
                        xh.append(xc)
                if st == 0:
                    # rope tables ride the hw queues behind strip 0's x
                    # chunks (the sw-DGE gpsimd queue is busy with wq until
                    # well after the first rope needs them)
                    nc.sync.dma_start(out=cq_sb, in_=cq_d[:, :])
                    nc.scalar.dma_start(out=sq_sb, in_=sq_d[:, :])
                    nc.sync.dma_start(out=ck_sb, in_=ck_d[:, :])
                    nc.scalar.dma_start(out=sk_sb, in_=sk_d[:, :])

                # ---- fused QKV projection + RoPE + v transpose ----
                def emit_jt(jt, xh_):
                    ps = pmm.tile([128, CW], f32, tag="mm", name="mm_ps")
                    for et in range(NET):
                        nc.tensor.matmul(
                            ps,
                            lhsT=wq_sb[jt][:, et, :],
                            rhs=xh_[et // ETC][:, et % ETC, :],
                            start=(et == 0),
                            stop=(et == NET - 1),
                        )
                    stx = stp.tile([128, CW], fp16, tag="st", name="stx")
                    nc.scalar.copy(stx, ps)
                    return stx

                def jt_stream(jt, xh_):
                    """emit_jt as a generator, one matmul per pull, so the
                    proj can fill exp-latency bubbles inside a QK/AV chain"""
                    ps = pmm.tile([128, CW], f32, tag="mm", name="mm_ps")
                    for et in range(NET):
                        nc.tensor.matmul(
                            ps,
                            lhsT=wq_sb[jt][:, et, :],
                            rhs=xh_[et // ETC][:, et % ETC, :],
                            start=(et == 0),
                            stop=(et == NET - 1),
                        )
                        yield
                    stx = stp.tile([128, CW], fp16, tag="st", name="stx")
                    nc.scalar.copy(stx, ps)
                    pre_stage.append(stx)

                if st == 1:
                    stage = pre_stage + [emit_jt(jt, xh) for jt in (4, 5)]
                else:
                    stage = [emit_jt(jt, xh) for jt in range(NJT)]

                q_sb = qp.tile([128, HPC, CW], fp16, tag="q")
                for sub in range(QPK + 1):     # RoPE on 4 q tiles + 1 k tile
                    stq = stage[sub]
                    is_q = sub < QPK
                    # interleaved pair-swap via partition-strided DMA
                    sw = rtp.tile([128, CW], fp16, tag="sw")
                    nc.sync.dma_start(out=sw[0::2, :], in_=stq[1::2, :])
                    nc.sync.dma_start(out=sw[1::2, :], in_=stq[0::2, :])
                    tmp = rtp.tile([128, CW], fp16, tag="rt")
                    nc.vector.tensor_mul(
                        tmp, sw, (sq_sb if is_q else sk_sb)[:, csl])
                    nc.vector.tensor_mul(
                        stq, stq, (cq_sb if is_q else ck_sb)[:, csl])
                    if is_q:
                        nc.vector.tensor_add(q_sb[:, sub, :], stq, tmp)
                    else:
                        nc.vector.tensor_add(k_sb[b][:, csl], stq, tmp)
                stv = stage[5]
                for u in range(CW // 128):
                    tp_ = pmm.tile([128, CW], fp16, tag="mm", name="tp_ps")
                    nc.tensor.transpose(
                        tp_[:, :128], stv[:, u * 128:(u + 1) * 128], ident
                    )
                    nc.scalar.copy(v_sb[b][:, (CW // 128) * c + u, :],
                                   tp_[:, :128])

                if st == 0:
                    # prefetch strip 1's x now: its q-head projections are
                    # streamed into strip 0's attention below (there are no
                    # wo blocks yet to keep the PE fed through the exp lag)
                    x1h = []
                    for ch in range(XCH):
                        xc = xtp.tile([128, ETC, CW], fp16, tag="xt",
                                      name="xc1")
                        eng = nc.sync if ch % 2 == 0 else nc.scalar
                        eng.dma_start(
                            out=xc,
                            in_=xt_d[0, 1, :, ch * ETC:(ch + 1) * ETC, :])
                        x1h.append(xc)
                    pre_stage = []

                # ---- attention for strip (b, c), flash-style, no max ----
                njt2 = (CW // 128) * (c + 1)   # causal: k tiles 0..4c+3
                ctx_sb = ctxp.tile([128, HPC, CW], fp16, tag="ctx")
                for h in range(HPC):
                    filler = jt_stream(h, x1h) if st == 0 else None
                    if st > 0:
                        # software pipeline: prev strip's output-projection
                        # blocks (2 per head) fill PE while ACT/DVE run softmax
                        pb, pc = divmod(st - 1, SC)
                        emit_wo_block(pb, pc, 2 * h, prev_ctx)
                        emit_wo_block(pb, pc, 2 * h + 1, prev_ctx)
                    ctx_ps = pacc.tile([128, CW], f32, tag="acc")
                    sums_ps = pacc.tile([128, CW], f32, tag="acc")
                    at_acc = None
                    at_prev = None
                    for j2 in range(njt2):
                        # diagonal k-tiles: trim the fully-masked columns
                        # from the QK matmul, exp AND attn@v; zero-fill the
                        # attn tile so the row-sums stay full-width
                        diag = j2 >= njt2 - (CW // 128)
                        o = 128 * (j2 - (njt2 - (CW // 128))) if diag else 0
                        nw = CW - o
                        qk = pqk.tile([128, CW], f32, tag="qk")
                        nc.tensor.matmul(
                            qk[:, :nw],
                            lhsT=k_sb[b][:, j2 * 128:(j2 + 1) * 128],
                            rhs=q_sb[:, h, o:],
                            start=True, stop=True,
                        )
                        if filler is not None:
                            # stream 8 of strip 1's proj matmuls between this
                            # QK and its AV so the PE rides through exp latency
                            for _ in range(NET // njt2):
                                next(filler, None)
                        at = atp.tile([128, CW], fp16, tag="at")
                        if o:
                            nc.gpsimd.memset(at[:, :o], 0.0)
                        nc.scalar.activation(
                            at[:, o:], qk[:, :nw],
                            mybir.ActivationFunctionType.Exp,
                            bias=bias_sb,
                        )
                        if diag:
                            nc.vector.tensor_mul(
                                at[:, o:o + 128], at[:, o:o + 128], mk_sb
                            )
                        first, last = j2 == 0, j2 == njt2 - 1
                        nc.tensor.matmul(
                            ctx_ps[:, o:], lhsT=v_sb[b][:, j2, :],
                            rhs=at[:, o:],
                            start=first, stop=last,
                            skip_group_check=bool(o),
                        )
                        # batch the row-sum matmul over ALL attn tiles of
                        # the head: accumulate on DVE (fp16, peak value
                        # ~16*e^(9-2) stays in range), one ones-matmul total
                        if j2 == 0:
                            at_prev = at
                        elif j2 == 1:
                            at_acc = atap.tile([128, CW], fp16, tag="ata",
                                               name="at_acc")
                            nc.vector.tensor_add(at_acc, at_prev, at)
                        else:
                            nc.vector.tensor_add(at_acc, at_acc, at)
                        if last:
                            nc.tensor.matmul(
                                sums_ps,
                                lhsT=ones_sb,
                                rhs=at_acc if njt2 > 1 else at_prev,
                                start=True, stop=True,
                            )
                    if filler is not None:
                        for _ in filler:
                            pass
                    rc = rcp.tile([128, CW], f32, tag="rc")
                    nc.vector.reciprocal_approx_fast(out=rc, in_=sums_ps)
                    nc.vector.tensor_mul(ctx_sb[:, h, :], ctx_ps, rc)

                prev_ctx = ctx_sb

            # drain: output projection for the final strip
            for ec in range(ECN):
                emit_wo_block(B - 1, SC - 1, ec, prev_ctx)
    nc.finalize()
    return nc


def _rope_tables(scale):
    inv = 1.0 / (ROPE_BASE ** (np.arange(0, D, 2, dtype=np.float64) / D))
    ang = np.arange(S, dtype=np.float64)[None, :] * inv[:, None]    # [D/2, S]
    C = np.empty((D, S), np.float32)
    Sx = np.empty((D, S), np.float32)
    C[0::2] = np.cos(ang)
    C[1::2] = np.cos(ang)
    Sx[0::2] = -np.sin(ang)
    Sx[1::2] = np.sin(ang)
    return (C * scale).astype(np_fp16), (Sx * scale).astype(np_fp16)


def _host_inputs(x, wqkv, wo):
    """Shard + retile inputs for the 8 cores. Core t = kv group t."""
    cq, sq = _rope_tables(D ** -0.5)
    ck, sk = _rope_tables(1.0)

    # causal boundary-block mask in scores^T layout: keep when jj <= ii
    jj = np.arange(128)[:, None]
    ii = np.arange(128)[None, :]
    mk = (jj <= ii).astype(np_fp16)

    xts = np.empty((B, SC, 128, NET, CW), np_fp16)
    for b in range(B):
        xT = np.ascontiguousarray(x[b].T)                 # [E, S]
        t = xT.reshape(NET, 128, SC, CW).transpose(2, 1, 0, 3)
        xts[b] = t.astype(np_fp16)

    rows = (E + 2 * KVH * D) // TP                        # 768
    in_maps = []
    for t in range(TP):
        wT = np.ascontiguousarray(wqkv[rows * t:rows * (t + 1)].T)  # [E, 768]
        wq_t = wT.reshape(NET, 128, NJT, 128).transpose(2, 1, 0, 3)
        woT = np.ascontiguousarray(wo[:, 512 * t:512 * (t + 1)].T)  # [512, E]
        wo_t = woT.reshape(FT, 128, ECN, CW).transpose(2, 1, 0, 3)
        in_maps.append({
            "xt": xts,
            "wq": np.ascontiguousarray(wq_t.astype(np_fp16)),
            "wo": np.ascontiguousarray(wo_t.astype(np_fp16)),
            "cq": cq, "sq": sq, "ck": ck, "sk": sk,
            "mk": mk,
        })
    return in_maps


def kernel(x, wqkv, wo):
    x = np.asarray(x, np.float32)
    wqkv = np.asarray(wqkv, np.float32)
    wo = np.asarray(wo, np.float32)

    if "nc" not in _built:
        _built["nc"] = _build_nc()
    nc = _built["nc"]

    in_maps = _host_inputs(x, wqkv, wo)
    res = run_bass_kernel_spmd(nc, in_maps, core_ids=list(range(NCORES)))
    globals()["_last_results"] = res

    acc = np.zeros((B, SC, ECN, 128, SC, CW), np.float32)
    for t in range(NCORES):
        acc += res.results[t]["out"].astype(np.float32)
    # [B, c, ec, p, sti, w] -> s = 512c + 128 sti + p, e = 512 ec + w
    out = acc.transpose(0, 1, 4, 3, 2, 5).reshape(B, S, E)
    return np.ascontiguousarray(out)
